# revision 11
# baseline (speedup 1.0000x reference)
"""MoE runtime-experts kernel for 8 Trainium2 NeuronCores.

Problem: y[t] = gelu(x[t] @ W1[e] + b1[e]) @ W2[e] + b2[e], e = indices[t].
T=8192 tokens, D=1024, H=4096, E=8 experts.

Strategy: expert-parallel. Host routes tokens by expert (argsort), core e
gets expert e's weights plus its tokens (transposed, zero-padded to a
common Tp so all 8 cores run one SPMD program). On device each core runs a
dense 2-layer MLP with fp32 PSUM accumulation:

  layer 1: hT[h, t] = gelu(sum_d W1[d, h] * xT[d, t] + b1[h])
           (lhsT = W1 k-tile [128d, 128h], rhs = xT [128d, 384t])
  layer 2: yT[d, t] = sum_h W2[h, d] * hT[h, t] + b2[d]
           (lhsT = W2 h-tile [128h, 128d], rhs = hT [128h, 384t])

Both layers keep the token axis in the free dimension, so no on-device
transpose is needed anywhere — and because tokens are always a free dim,
Tp needs no alignment: every core computes exactly max(counts) token
columns, split into balanced chunks of <=384 (one fp32 PSUM bank each).
Token-chunk DMAs are spread across the sync and gpsimd rings while the
scalar ring streams w1, so the PE starts ~13 us in and stays >=90% busy.
Host un-permutes yT shards into the full [T, 1, D] output.

KERNEL_MODE selects compute dtype: "fp8s" (default; both layers fp8e4m3
+ DoubleRow, with W1 sent as W1-0.5 and the exact rank-1 correction
c[t] = 0.5*sum_d x[d,t] computed on host in fp32 and added on-device by
the vector engine before gelu — this removes the common-mode error of
naive fp8 that fails the 2e-2 gate), "bf16", "fp8" (naive fp8, fails
the gate), "fp8l1" (layer 1 fp8, layer 2 bf16).
"""

import math
import os

import numpy as np
import ml_dtypes

T, D, H, E = 8192, 1024, 4096, 8
N_CORES = 8
KB_D = D // 128  # 8  k-tiles of the D contraction
HB = H // 128  # 32 h-tiles
DB = D // 128  # 8  d-tiles
BF16 = ml_dtypes.bfloat16
CS = 384  # token chunk (matmul moving-operand free dim)
SUP = 4 * CS  # tokens resident per pass (SBUF limit)
MM_N = 512  # PSUM bank free size (fp32)

MODE = os.environ.get("KERNEL_MODE", "fp8s")

_program_cache: dict[tuple, object] = {}
last_results = None  # BassKernelResults of the most recent kernel() call


def _chunk_sizes(Tp: int):
    """Balanced split of Tp token columns into chunks of at most CS."""
    nch = max(1, math.ceil(Tp / CS))
    base, rem = divmod(Tp, nch)
    return [base + (1 if i < rem else 0) for i in range(nch)]


def _build_program(Tp: int, mode: str):
    import concourse.tile as tile
    from concourse import bacc, mybir

    sizes = _chunk_sizes(Tp)
    nch = len(sizes)
    offs = [sum(sizes[:i]) for i in range(nch)]  # global token offsets

    f32 = mybir.dt.float32
    bf16 = mybir.dt.bfloat16
    fp8 = mybir.dt.float8e4
    l1_dt = fp8 if mode in ("fp8", "fp8l1", "fp8s") else bf16
    l2_dt = fp8 if mode in ("fp8", "fp8s") else bf16
    shifted = mode == "fp8s"
    l1_dr = l1_dt == fp8
    l2_dr = l2_dt == fp8
    dr = mybir.MatmulPerfMode.DoubleRow
    gelu = mybir.ActivationFunctionType.Gelu
    ident = mybir.ActivationFunctionType.Identity

    nc = bacc.Bacc(
        "TRN2", target_bir_lowering=False, debug=False, num_devices=N_CORES
    )

    # xq[c] is the SBUF image of token chunk c: [128, KB_D*CS], row-major
    # (kb, t) per partition, so the DMA is fully contiguous
    xq = nc.dram_tensor(
        "xq", [nch, 128, KB_D * CS], l1_dt, kind="ExternalInput"
    ).ap()
    # w1[h] is a [128, KB_D*128] block: col-chunk kb holds W1[kb*128+p, h*128+m]
    w1 = nc.dram_tensor(
        "w1", [HB, 128, KB_D * 128], l1_dt, kind="ExternalInput"
    ).ap()
    # w2[d] is a [128, HB*128] block: col-chunk hb holds W2[hb*128+p, d*128+m]
    w2 = nc.dram_tensor(
        "w2", [DB, 128, HB * 128], l2_dt, kind="ExternalInput"
    ).ap()
    b1 = nc.dram_tensor("b1", [128, HB], f32, kind="ExternalInput").ap()
    b2 = nc.dram_tensor("b2", [128, DB], f32, kind="ExternalInput").ap()
    # cq[c] = 0.5*colsum(x) for chunk c's tokens, replicated over the 128
    # partitions (fp32; the rank-1 mean-shift correction for fp8s mode)
    cq = (
        nc.dram_tensor("cq", [nch, 128, CS], f32, kind="ExternalInput").ap()
        if shifted
        else None
    )
    yT = nc.dram_tensor("yT", [D, Tp], f32, kind="ExternalOutput").ap()

    def mm_group(ps, tsz, nk, lhs_of, rhs_of, use_dr):
        """Accumulate nk k-tiles into psum ps[:, :tsz]; DoubleRow fuses
        pairs of k-tiles per matmul via 3D APs."""
        if use_dr:
            for j in range(0, nk, 2):
                nc.tensor.matmul(
                    ps[:, :tsz],
                    lhs_of(j, 2),
                    rhs_of(j, 2),
                    start=(j == 0),
                    stop=(j == nk - 2),
                    perf_mode=dr,
                )
        else:
            for j in range(nk):
                nc.tensor.matmul(
                    ps[:, :tsz],
                    lhs_of(j, 1),
                    rhs_of(j, 1),
                    start=(j == 0),
                    stop=(j == nk - 1),
                )

    with tile.TileContext(nc) as tc:
        with (
            tc.tile_pool(name="const", bufs=1) as const_pool,
            tc.tile_pool(name="acts", bufs=1) as acts_pool,
            tc.tile_pool(name="xtp", bufs=3) as xt_pool,
            tc.tile_pool(name="w1p", bufs=4) as w1_pool,
            tc.tile_pool(name="w2p", bufs=2) as w2_pool,
            tc.tile_pool(name="outp", bufs=4) as out_pool,
            tc.tile_pool(name="psum", bufs=8, space="PSUM") as psum_pool,
        ):
            b1_sb = const_pool.tile([128, HB], f32)
            b2_sb = const_pool.tile([128, DB], f32)

            for sup0 in range(0, nch, SUP // CS):

                cix = list(range(sup0, min(sup0 + SUP // CS, nch)))
                loffs = [offs[c] - offs[cix[0]] for c in cix]  # ht-local
                sup_len = sum(sizes[c] for c in cix)
                ht_sb = acts_pool.tile([128, HB, sup_len], l2_dt, tag="ht")

                # token chunks: chunk 0 on the sync ring (gates the first
                # matmul), the rest on the gpsimd ring in parallel; the
                # scalar ring carries only the w1 stream
                xts = []
                cqs = []
                for ci, c in enumerate(cix):
                    xt_c = xt_pool.tile(
                        [128, KB_D, CS], l1_dt, tag=f"xt{ci}", bufs=1
                    )
                    eng = nc.sync if ci == 0 else nc.gpsimd
                    eng.dma_start(
                        xt_c[:], xq[c].rearrange("p (k m) -> p k m", k=KB_D)
                    )
                    xts.append(xt_c)
                    if shifted:
                        cq_c = xt_pool.tile(
                            [128, CS], f32, tag=f"cq{ci}", bufs=1
                        )
                        (nc.sync if ci == 0 else nc.gpsimd).dma_start(
                            cq_c[:], cq[c]
                        )
                        cqs.append(cq_c)
                if sup0 == 0:
                    nc.sync.dma_start(b1_sb[:], b1[:])
                    nc.sync.dma_start(b2_sb[:], b2[:])

                # ---- layer 1: hT[h, c] ----
                for h in range(HB):
                    w1t = w1_pool.tile([128, KB_D, 128], l1_dt, tag="w1t")
                    nc.scalar.dma_start(
                        w1t[:], w1[h].rearrange("p (k m) -> p k m", k=KB_D)
                    )
                    for ci, c in enumerate(cix):
                        xt_c = xts[ci]
                        tsz = sizes[c]
                        lo = loffs[ci]
                        ps = psum_pool.tile([128, MM_N], f32, tag="ps")
                        mm_group(
                            ps,
                            tsz,
                            KB_D,
                            lambda j, w: w1t[:, j : j + w, :]
                            if w == 2
                            else w1t[:, j, :],
                            lambda j, w: xt_c[:, j : j + w, :tsz]
                            if w == 2
                            else xt_c[:, j, :tsz],
                            l1_dr,
                        )
                        if shifted:
                            # psum += c (per-token rank-1 mean correction)
                            nc.vector.scalar_tensor_tensor(
                                ps[:, :tsz],
                                ps[:, :tsz],
                                1.0,
                                cqs[ci][:, :tsz],
                                mybir.AluOpType.mult,
                                mybir.AluOpType.add,
                            )
                        nc.scalar.activation(
                            ht_sb[:, h, lo : lo + tsz],
                            ps[:, :tsz],
                            gelu,
                            bias=b1_sb[:, h : h + 1],
                        )

                # ---- layer 2: yT[d, c] ----
                for d in range(DB):
                    # w2 on the gpsimd (SWDGE) ring: parallel to the w1
                    # stream on the scalar ring, so d=0 prefetches early
                    w2t = w2_pool.tile([128, HB, 128], l2_dt, tag="w2t")
                    nc.gpsimd.dma_start(
                        w2t[:], w2[d].rearrange("p (k m) -> p k m", k=HB)
                    )
                    for ci, c in enumerate(cix):
                        tsz = sizes[c]
                        lo = loffs[ci]
                        go = offs[c]
                        ps = psum_pool.tile([128, MM_N], f32, tag="ps")
                        mm_group(
                            ps,
                            tsz,
                            HB,
                            lambda j, w: w2t[:, j : j + w, :]
                            if w == 2
                            else w2t[:, j, :],
                            lambda j, w: ht_sb[:, j : j + w, lo : lo + tsz]
                            if w == 2
                            else ht_sb[:, j, lo : lo + tsz],
                            l2_dr,
                        )
                        ot = out_pool.tile([128, MM_N], f32, tag="ot")
                        # final store: split so the exposed ACT+DMA tail
                        # after the last matmul shrinks
                        last = d == DB - 1 and c == cix[-1]
                        pieces = (
                            [(0, tsz - 128), (tsz - 128, 128)]
                            if last and tsz > 256
                            else [(0, tsz)]
                        )
                        # the final d-iteration's stores ride the scalar
                        # ring, which is idle by then — the sync ring may
                        # still be draining earlier output stores
                        st_eng = nc.scalar if d == DB - 1 else nc.sync
                        for p0, psz in pieces:
                            nc.scalar.activation(
                                ot[:, p0 : p0 + psz],
                                ps[:, p0 : p0 + psz],
                                ident,
                                bias=b2_sb[:, d : d + 1],
                            )
                            st_eng.dma_start(
                                yT[
                                    d * 128 : (d + 1) * 128,
                                    go + p0 : go + p0 + psz,
                                ],
                                ot[:, p0 : p0 + psz],
                            )

    nc.compile()
    return nc


def kernel(x, indices_s, weight1, weight2, bias1, bias2):
    from concourse import mybir
    from concourse.bass_utils import run_bass_kernel_spmd

    x = np.asarray(x, dtype=np.float32)
    idx = np.asarray(indices_s).astype(np.int64).ravel()
    w1_full = np.asarray(weight1, dtype=np.float32)
    w2_full = np.asarray(weight2, dtype=np.float32)
    b1_full = np.asarray(bias1, dtype=np.float32)
    b2_full = np.asarray(bias2, dtype=np.float32)

    order = np.argsort(idx, kind="stable")
    counts = np.bincount(idx, minlength=E)
    starts = np.concatenate([[0], np.cumsum(counts)])
    # tokens live in the free dim everywhere, so no alignment is needed:
    # every core computes exactly max(counts) token columns
    Tp = max(128, int(counts.max()))
    sizes = _chunk_sizes(Tp)
    nch = len(sizes)
    offs = np.concatenate([[0], np.cumsum(sizes)])

    mode = MODE
    key = (Tp, mode)
    nc = _program_cache.get(key)
    if nc is None:
        nc = _build_program(Tp, mode)
        _program_cache[key] = nc

    fp8_np = mybir.dt.np(mybir.dt.float8e4)
    l1_np = fp8_np if mode in ("fp8", "fp8l1", "fp8s") else BF16
    l2_np = fp8_np if mode in ("fp8", "fp8s") else BF16
    shifted = mode == "fp8s"
    w1_shift = np.float32(0.5) if shifted else np.float32(0.0)

    in_maps = []
    for e in range(E):
        toks = order[starts[e] : starts[e + 1]]
        # slot-aligned image: chunk c's tokens at columns [c*CS, c*CS+sizes[c])
        xTs = np.zeros((D, nch * CS), dtype=np.float32)
        for c in range(nch):
            lo, hi = offs[c], min(offs[c + 1], counts[e])
            if hi > lo:
                xTs[:, c * CS : c * CS + (hi - lo)] = x[toks[lo:hi]].T
        # [D, nch*CS] -> [nch, 128, KB_D*CS] chunk-major SBUF image
        xq = (
            np.ascontiguousarray(
                xTs.reshape(KB_D, 128, nch, CS).transpose(2, 1, 0, 3)
            )
            .reshape(nch, 128, KB_D * CS)
            .astype(l1_np)
        )
        w1r = (
            np.ascontiguousarray(
                (w1_full[e] - w1_shift)
                .reshape(KB_D, 128, HB, 128)
                .transpose(2, 1, 0, 3)
            )
            .reshape(HB, 128, KB_D * 128)
            .astype(l1_np)
        )
        w2r = (
            np.ascontiguousarray(
                w2_full[e].reshape(HB, 128, DB, 128).transpose(2, 1, 0, 3)
            )
            .reshape(DB, 128, HB * 128)
            .astype(l2_np)
        )
        b1d = np.ascontiguousarray(b1_full[e].reshape(HB, 128).T)
        b2d = np.ascontiguousarray(b2_full[e].reshape(DB, 128).T)
        im = {"xq": xq, "w1": w1r, "w2": w2r, "b1": b1d, "b2": b2d}
        if shifted:
            # c[t] = 0.5 * sum_d x[t, d] in fp32, slot-aligned like xq,
            # replicated across the 128 partitions
            cvals = np.zeros((nch * CS,), dtype=np.float32)
            for c in range(nch):
                lo, hi = offs[c], min(offs[c + 1], counts[e])
                if hi > lo:
                    cvals[c * CS : c * CS + (hi - lo)] = (
                        0.5 * x[toks[lo:hi]].sum(axis=1)
                    )
            im["cq"] = np.ascontiguousarray(
                np.broadcast_to(
                    cvals.reshape(nch, 1, CS), (nch, 128, CS)
                )
            )
        in_maps.append(im)

    res = run_bass_kernel_spmd(
        nc,
        in_maps,
        list(range(N_CORES)),
        trace=os.environ.get("BASS_TRACE") == "1",
    )
    global last_results
    last_results = res

    out = np.empty((T, D), dtype=np.float32)
    for e in range(E):
        toks = order[starts[e] : starts[e + 1]]
        out[toks] = res.results[e]["yT"][:, : counts[e]].T
    if res.exec_time_ns is not None:
        print(f"HW exec time: {res.exec_time_ns} ns")
    return out[:, None, :]



# revision 14
# speedup vs baseline: 1.2051x; 1.2051x over previous
"""MoE runtime-experts kernel for 8 Trainium2 NeuronCores.

Problem: y[t] = gelu(x[t] @ W1[e] + b1[e]) @ W2[e] + b2[e], e = indices[t].
T=8192 tokens, D=1024, H=4096, E=8 experts.

Strategy: expert-parallel. Host routes tokens by expert (argsort), core e
gets expert e's weights plus its tokens (transposed, zero-padded to a
common Tp so all 8 cores run one SPMD program). On device each core runs a
dense 2-layer MLP with fp32 PSUM accumulation:

  layer 1: hT[h, t] = gelu(sum_d W1[d, h] * xT[d, t] + b1[h])
           (lhsT = W1 k-tile [128d, 128h], rhs = xT [128d, 384t])
  layer 2: yT[d, t] = sum_h W2[h, d] * hT[h, t] + b2[d]
           (lhsT = W2 h-tile [128h, 128d], rhs = hT [128h, 384t])

Both layers keep the token axis in the free dimension, so no on-device
transpose is needed anywhere — and because tokens are always a free dim,
Tp needs no alignment: every core computes exactly max(counts) token
columns, split into balanced chunks of <=384 (one fp32 PSUM bank each).
Token-chunk DMAs are spread across the sync and gpsimd rings while the
scalar ring streams w1, so the PE starts ~13 us in and stays >=90% busy.
Host un-permutes yT shards into the full [T, 1, D] output.

KERNEL_MODE selects compute dtype: "fp8s" (default; both layers fp8e4m3
+ DoubleRow, with W1 sent as W1-0.5 and the exact rank-1 correction
c[t] = 0.5*sum_d x[d,t] computed on host in fp32 and added on-device by
the vector engine before gelu — this removes the common-mode error of
naive fp8 that fails the 2e-2 gate), "bf16", "fp8" (naive fp8, fails
the gate), "fp8l1" (layer 1 fp8, layer 2 bf16).
"""

import math
import os

import numpy as np
import ml_dtypes

T, D, H, E = 8192, 1024, 4096, 8
N_CORES = 8
KB_D = D // 128  # 8  k-tiles of the D contraction
HB = H // 128  # 32 h-tiles
DB = D // 128  # 8  d-tiles
BF16 = ml_dtypes.bfloat16
CS = 384  # token chunk (matmul moving-operand free dim)
SUP = 4 * CS  # tokens resident per pass (SBUF limit)
MM_N = 512  # PSUM bank free size (fp32)

MODE = os.environ.get("KERNEL_MODE", "fp8s")

_program_cache: dict[tuple, object] = {}
last_results = None  # BassKernelResults of the most recent kernel() call


def _chunk_sizes(Tp: int):
    """Balanced split of Tp token columns into chunks of at most CS."""
    nch = max(1, math.ceil(Tp / CS))
    base, rem = divmod(Tp, nch)
    return [base + (1 if i < rem else 0) for i in range(nch)]


def _build_program(Tp: int, mode: str):
    import concourse.tile as tile
    from concourse import bacc, mybir

    sizes = _chunk_sizes(Tp)
    nch = len(sizes)
    offs = [sum(sizes[:i]) for i in range(nch)]  # global token offsets

    f32 = mybir.dt.float32
    bf16 = mybir.dt.bfloat16
    fp8 = mybir.dt.float8e4
    l1_dt = fp8 if mode in ("fp8", "fp8l1", "fp8s") else bf16
    l2_dt = fp8 if mode in ("fp8", "fp8s") else bf16
    shifted = mode == "fp8s"
    l1_dr = l1_dt == fp8
    l2_dr = l2_dt == fp8
    dr = mybir.MatmulPerfMode.DoubleRow
    gelu = mybir.ActivationFunctionType.Gelu
    ident = mybir.ActivationFunctionType.Identity

    nc = bacc.Bacc(
        "TRN2", target_bir_lowering=False, debug=False, num_devices=N_CORES
    )

    # xq[c] is the SBUF image of token chunk c: [128, KB_D*CS], row-major
    # (kb, t) per partition, so the DMA is fully contiguous
    xq = nc.dram_tensor(
        "xq", [nch, 128, KB_D * CS], l1_dt, kind="ExternalInput"
    ).ap()
    # w1[h] is a [128, KB_D*128] block: col-chunk kb holds W1[kb*128+p, h*128+m]
    w1 = nc.dram_tensor(
        "w1", [HB, 128, KB_D * 128], l1_dt, kind="ExternalInput"
    ).ap()
    # w2[d] is a [128, HB*128] block: col-chunk hb holds W2[hb*128+p, d*128+m]
    w2 = nc.dram_tensor(
        "w2", [DB, 128, HB * 128], l2_dt, kind="ExternalInput"
    ).ap()
    b1 = nc.dram_tensor("b1", [128, HB], f32, kind="ExternalInput").ap()
    b2 = nc.dram_tensor("b2", [128, DB], f32, kind="ExternalInput").ap()
    # cq[c] = 0.5*colsum(x) for chunk c's tokens, replicated over the 128
    # partitions (fp32; the rank-1 mean-shift correction for fp8s mode)
    cq = (
        nc.dram_tensor("cq", [nch, 128, CS], f32, kind="ExternalInput").ap()
        if shifted
        else None
    )
    yT = nc.dram_tensor("yT", [D, Tp], f32, kind="ExternalOutput").ap()

    def mm_group(ps, tsz, nk, lhs_of, rhs_of, use_dr):
        """Accumulate nk k-tiles into psum ps[:, :tsz]; DoubleRow fuses
        pairs of k-tiles per matmul via 3D APs."""
        if use_dr:
            for j in range(0, nk, 2):
                nc.tensor.matmul(
                    ps[:, :tsz],
                    lhs_of(j, 2),
                    rhs_of(j, 2),
                    start=(j == 0),
                    stop=(j == nk - 2),
                    perf_mode=dr,
                )
        else:
            for j in range(nk):
                nc.tensor.matmul(
                    ps[:, :tsz],
                    lhs_of(j, 1),
                    rhs_of(j, 1),
                    start=(j == 0),
                    stop=(j == nk - 1),
                )

    with tile.TileContext(nc) as tc:
        with (
            tc.tile_pool(name="const", bufs=1) as const_pool,
            tc.tile_pool(name="acts", bufs=1) as acts_pool,
            tc.tile_pool(name="xtp", bufs=3) as xt_pool,
            tc.tile_pool(name="w1p", bufs=4) as w1_pool,
            tc.tile_pool(name="w2p", bufs=2) as w2_pool,
            tc.tile_pool(name="outp", bufs=4) as out_pool,
            tc.tile_pool(name="psum", bufs=7, space="PSUM") as psum_pool,
            tc.tile_pool(name="warm", bufs=1, space="PSUM") as warm_pool,
        ):
            b1_sb = const_pool.tile([128, HB], f32)
            b2_sb = const_pool.tile([128, DB], f32)

            # HAM warmup: ~8 dummy matmuls fill the PE while the first
            # DMAs land, so real matmuls start at 2.4 GHz instead of 1.2
            warm_sb = const_pool.tile([128, MM_N], l1_dt)
            nc.vector.memset(warm_sb[:], 0.0)
            warm_ps = warm_pool.tile([128, MM_N], f32, tag="warm")
            for _ in range(8):
                nc.tensor.matmul(
                    warm_ps[:, :MM_N],
                    warm_sb[:, :128],
                    warm_sb[:, :MM_N],
                    start=True,
                    stop=True,
                )

            for sup0 in range(0, nch, SUP // CS):

                cix = list(range(sup0, min(sup0 + SUP // CS, nch)))
                loffs = [offs[c] - offs[cix[0]] for c in cix]  # ht-local
                sup_len = sum(sizes[c] for c in cix)
                ht_sb = acts_pool.tile([128, HB, sup_len], l2_dt, tag="ht")

                # token chunks: chunk 0 split per k-pair on the sync ring
                # (its first pair gates the first matmul), the rest whole
                # on the gpsimd ring; w1 rides the sync ring after them
                xts = []
                cqs = []
                for ci, c in enumerate(cix):
                    xt_c = xt_pool.tile(
                        [128, KB_D, CS], l1_dt, tag=f"xt{ci}", bufs=1
                    )
                    xr = xq[c].rearrange("p (k m) -> p k m", k=KB_D)
                    if ci == 0 and sup0 == 0:
                        for j in range(0, KB_D, 2):
                            nc.sync.dma_start(
                                xt_c[:, j : j + 2, :], xr[:, j : j + 2, :]
                            )
                    else:
                        nc.gpsimd.dma_start(xt_c[:], xr)
                    xts.append(xt_c)
                    if shifted:
                        cq_c = xt_pool.tile(
                            [128, CS], f32, tag=f"cq{ci}", bufs=1
                        )
                        (nc.sync if ci == 0 else nc.gpsimd).dma_start(
                            cq_c[:], cq[c]
                        )
                        cqs.append(cq_c)
                if sup0 == 0:
                    nc.sync.dma_start(b1_sb[:], b1[:])
                    nc.sync.dma_start(b2_sb[:], b2[:])

                # ---- layer 1: hT[h, c] ----
                for h in range(HB):
                    w1t = w1_pool.tile([128, KB_D, 128], l1_dt, tag="w1t")
                    w1r = w1[h].rearrange("p (k m) -> p k m", k=KB_D)
                    if h == 0 and sup0 == 0:
                        # first tile split per k-pair on the idle scalar
                        # ring so pair 0 lands with xt0's pair 0
                        for j in range(0, KB_D, 2):
                            nc.scalar.dma_start(
                                w1t[:, j : j + 2, :], w1r[:, j : j + 2, :]
                            )
                    else:
                        nc.sync.dma_start(w1t[:], w1r)
                    for ci, c in enumerate(cix):
                        xt_c = xts[ci]
                        tsz = sizes[c]
                        lo = loffs[ci]
                        ps = psum_pool.tile([128, MM_N], f32, tag="ps")
                        mm_group(
                            ps,
                            tsz,
                            KB_D,
                            lambda j, w: w1t[:, j : j + w, :]
                            if w == 2
                            else w1t[:, j, :],
                            lambda j, w: xt_c[:, j : j + w, :tsz]
                            if w == 2
                            else xt_c[:, j, :tsz],
                            l1_dr,
                        )
                        if shifted:
                            # psum += c (per-token rank-1 mean correction)
                            nc.vector.scalar_tensor_tensor(
                                ps[:, :tsz],
                                ps[:, :tsz],
                                1.0,
                                cqs[ci][:, :tsz],
                                mybir.AluOpType.mult,
                                mybir.AluOpType.add,
                            )
                        nc.scalar.activation(
                            ht_sb[:, h, lo : lo + tsz],
                            ps[:, :tsz],
                            gelu,
                            bias=b1_sb[:, h : h + 1],
                        )

                # ---- layer 2: yT[d, c] ----
                for d in range(DB):
                    # w2 on the gpsimd (SWDGE) ring: parallel to the w1
                    # stream on the scalar ring, so d=0 prefetches early
                    w2t = w2_pool.tile([128, HB, 128], l2_dt, tag="w2t")
                    nc.gpsimd.dma_start(
                        w2t[:], w2[d].rearrange("p (k m) -> p k m", k=HB)
                    )
                    for ci, c in enumerate(cix):
                        tsz = sizes[c]
                        lo = loffs[ci]
                        go = offs[c]
                        ps = psum_pool.tile([128, MM_N], f32, tag="ps")
                        mm_group(
                            ps,
                            tsz,
                            HB,
                            lambda j, w: w2t[:, j : j + w, :]
                            if w == 2
                            else w2t[:, j, :],
                            lambda j, w: ht_sb[:, j : j + w, lo : lo + tsz]
                            if w == 2
                            else ht_sb[:, j, lo : lo + tsz],
                            l2_dr,
                        )
                        ot = out_pool.tile([128, MM_N], f32, tag="ot")
                        if d == DB - 1:
                            # last d-tile: fan the final stores out over
                            # all three DMA rings, split in two pieces
                            # each, so the post-matmul tail is short
                            st_eng = [nc.scalar, nc.sync, nc.gpsimd][
                                ci % 3
                            ]
                            half = tsz // 2
                            pieces = [(0, half), (half, tsz - half)]
                        else:
                            # bulk stores alternate sync/gpsimd (both
                            # idle during layer 2; scalar runs the ACTs)
                            st_eng = nc.sync if ci % 2 == 0 else nc.gpsimd
                            pieces = [(0, tsz)]
                        for p0, psz in pieces:
                            nc.scalar.activation(
                                ot[:, p0 : p0 + psz],
                                ps[:, p0 : p0 + psz],
                                ident,
                                bias=b2_sb[:, d : d + 1],
                            )
                            st_eng.dma_start(
                                yT[
                                    d * 128 : (d + 1) * 128,
                                    go + p0 : go + p0 + psz,
                                ],
                                ot[:, p0 : p0 + psz],
                            )

    nc.compile()
    return nc


def kernel(x, indices_s, weight1, weight2, bias1, bias2):
    from concourse import mybir
    from concourse.bass_utils import run_bass_kernel_spmd

    x = np.asarray(x, dtype=np.float32)
    idx = np.asarray(indices_s).astype(np.int64).ravel()
    w1_full = np.asarray(weight1, dtype=np.float32)
    w2_full = np.asarray(weight2, dtype=np.float32)
    b1_full = np.asarray(bias1, dtype=np.float32)
    b2_full = np.asarray(bias2, dtype=np.float32)

    order = np.argsort(idx, kind="stable")
    counts = np.bincount(idx, minlength=E)
    starts = np.concatenate([[0], np.cumsum(counts)])
    # tokens live in the free dim everywhere, so no alignment is needed:
    # every core computes exactly max(counts) token columns
    Tp = max(128, int(counts.max()))
    sizes = _chunk_sizes(Tp)
    nch = len(sizes)
    offs = np.concatenate([[0], np.cumsum(sizes)])

    mode = MODE
    key = (Tp, mode)
    nc = _program_cache.get(key)
    if nc is None:
        nc = _build_program(Tp, mode)
        _program_cache[key] = nc

    fp8_np = mybir.dt.np(mybir.dt.float8e4)
    l1_np = fp8_np if mode in ("fp8", "fp8l1", "fp8s") else BF16
    l2_np = fp8_np if mode in ("fp8", "fp8s") else BF16
    shifted = mode == "fp8s"
    w1_shift = np.float32(0.5) if shifted else np.float32(0.0)

    in_maps = []
    for e in range(E):
        toks = order[starts[e] : starts[e + 1]]
        # slot-aligned image: chunk c's tokens at columns [c*CS, c*CS+sizes[c])
        xTs = np.zeros((D, nch * CS), dtype=np.float32)
        for c in range(nch):
            lo, hi = offs[c], min(offs[c + 1], counts[e])
            if hi > lo:
                xTs[:, c * CS : c * CS + (hi - lo)] = x[toks[lo:hi]].T
        # [D, nch*CS] -> [nch, 128, KB_D*CS] chunk-major SBUF image
        xq = (
            np.ascontiguousarray(
                xTs.reshape(KB_D, 128, nch, CS).transpose(2, 1, 0, 3)
            )
            .reshape(nch, 128, KB_D * CS)
            .astype(l1_np)
        )
        w1r = (
            np.ascontiguousarray(
                (w1_full[e] - w1_shift)
                .reshape(KB_D, 128, HB, 128)
                .transpose(2, 1, 0, 3)
            )
            .reshape(HB, 128, KB_D * 128)
            .astype(l1_np)
        )
        w2r = (
            np.ascontiguousarray(
                w2_full[e].reshape(HB, 128, DB, 128).transpose(2, 1, 0, 3)
            )
            .reshape(DB, 128, HB * 128)
            .astype(l2_np)
        )
        b1d = np.ascontiguousarray(b1_full[e].reshape(HB, 128).T)
        b2d = np.ascontiguousarray(b2_full[e].reshape(DB, 128).T)
        im = {"xq": xq, "w1": w1r, "w2": w2r, "b1": b1d, "b2": b2d}
        if shifted:
            # c[t] = 0.5 * sum_d x[t, d] in fp32, slot-aligned like xq,
            # replicated across the 128 partitions
            cvals = np.zeros((nch * CS,), dtype=np.float32)
            for c in range(nch):
                lo, hi = offs[c], min(offs[c + 1], counts[e])
                if hi > lo:
                    cvals[c * CS : c * CS + (hi - lo)] = (
                        0.5 * x[toks[lo:hi]].sum(axis=1)
                    )
            im["cq"] = np.ascontiguousarray(
                np.broadcast_to(
                    cvals.reshape(nch, 1, CS), (nch, 128, CS)
                )
            )
        in_maps.append(im)

    res = run_bass_kernel_spmd(
        nc,
        in_maps,
        list(range(N_CORES)),
        trace=os.environ.get("BASS_TRACE") == "1",
    )
    global last_results
    last_results = res

    out = np.empty((T, D), dtype=np.float32)
    for e in range(E):
        toks = order[starts[e] : starts[e + 1]]
        out[toks] = res.results[e]["yT"][:, : counts[e]].T
    if res.exec_time_ns is not None:
        print(f"HW exec time: {res.exec_time_ns} ns")
    return out[:, None, :]



# revision 18
# speedup vs baseline: 1.2159x; 1.0090x over previous
"""MoE runtime-experts kernel for 8 Trainium2 NeuronCores.

Problem: y[t] = gelu(x[t] @ W1[e] + b1[e]) @ W2[e] + b2[e], e = indices[t].
T=8192 tokens, D=1024, H=4096, E=8 experts.

Strategy: expert-parallel. Host routes tokens by expert (argsort), core e
gets expert e's weights plus its tokens (transposed, zero-padded to a
common Tp so all 8 cores run one SPMD program). On device each core runs a
dense 2-layer MLP with fp32 PSUM accumulation:

  layer 1: hT[h, t] = gelu(sum_d W1[d, h] * xT[d, t] + b1[h])
           (lhsT = W1 k-tile [128d, 128h], rhs = xT [128d, 384t])
  layer 2: yT[d, t] = sum_h W2[h, d] * hT[h, t] + b2[d]
           (lhsT = W2 h-tile [128h, 128d], rhs = hT [128h, 384t])

Both layers keep the token axis in the free dimension, so no on-device
transpose is needed anywhere — and because tokens are always a free dim,
Tp needs no alignment: every core computes exactly max(counts) token
columns, split into balanced chunks of <=384 (one fp32 PSUM bank each).
Token-chunk DMAs are spread across the sync and gpsimd rings while the
scalar ring streams w1, so the PE starts ~13 us in and stays >=90% busy.
Host un-permutes yT shards into the full [T, 1, D] output.

KERNEL_MODE selects compute dtype: "fp8s" (default; both layers fp8e4m3
+ DoubleRow, with W1 sent as W1-0.5 and the exact rank-1 correction
c[t] = 0.5*sum_d x[d,t] computed on host in fp32 and added on-device by
the vector engine before gelu — this removes the common-mode error of
naive fp8 that fails the 2e-2 gate), "bf16", "fp8" (naive fp8, fails
the gate), "fp8l1" (layer 1 fp8, layer 2 bf16).
"""

import math
import os

import numpy as np
import ml_dtypes

T, D, H, E = 8192, 1024, 4096, 8
N_CORES = 8
KB_D = D // 128  # 8  k-tiles of the D contraction
HB = H // 128  # 32 h-tiles
DB = D // 128  # 8  d-tiles
BF16 = ml_dtypes.bfloat16
CS = 384  # token chunk (matmul moving-operand free dim)
SUP = 4 * CS  # tokens resident per pass (SBUF limit)
MM_N = 512  # PSUM bank free size (fp32)

MODE = os.environ.get("KERNEL_MODE", "fp8s")

_program_cache: dict[tuple, object] = {}
last_results = None  # BassKernelResults of the most recent kernel() call


def _chunk_sizes(Tp: int):
    """Balanced split of Tp token columns into chunks of at most CS."""
    nch = max(1, math.ceil(Tp / CS))
    base, rem = divmod(Tp, nch)
    return [base + (1 if i < rem else 0) for i in range(nch)]


def _build_program(Tp: int, mode: str):
    import concourse.tile as tile
    from concourse import bacc, mybir

    sizes = _chunk_sizes(Tp)
    nch = len(sizes)
    offs = [sum(sizes[:i]) for i in range(nch)]  # global token offsets

    f32 = mybir.dt.float32
    bf16 = mybir.dt.bfloat16
    fp8 = mybir.dt.float8e4
    l1_dt = fp8 if mode in ("fp8", "fp8l1", "fp8s") else bf16
    l2_dt = fp8 if mode in ("fp8", "fp8s") else bf16
    shifted = mode == "fp8s"
    l1_dr = l1_dt == fp8
    l2_dr = l2_dt == fp8
    dr = mybir.MatmulPerfMode.DoubleRow
    gelu = mybir.ActivationFunctionType.Gelu
    ident = mybir.ActivationFunctionType.Identity

    nc = bacc.Bacc(
        "TRN2", target_bir_lowering=False, debug=False, num_devices=N_CORES
    )

    # xq[c] is the SBUF image of token chunk c: [128, KB_D*CS], row-major
    # (kb, t) per partition, so the DMA is fully contiguous
    xq = nc.dram_tensor(
        "xq", [nch, 128, KB_D * CS], l1_dt, kind="ExternalInput"
    ).ap()
    # w1[h] is a [128, KB_D*128] block: col-chunk kb holds W1[kb*128+p, h*128+m]
    w1 = nc.dram_tensor(
        "w1", [HB, 128, KB_D * 128], l1_dt, kind="ExternalInput"
    ).ap()
    # w2[d] is a [128, HB*128] block: col-chunk hb holds W2[hb*128+p, d*128+m]
    w2 = nc.dram_tensor(
        "w2", [DB, 128, HB * 128], l2_dt, kind="ExternalInput"
    ).ap()
    b1 = nc.dram_tensor("b1", [128, HB], f32, kind="ExternalInput").ap()
    b2 = nc.dram_tensor("b2", [128, DB], f32, kind="ExternalInput").ap()
    # cq[c] = 0.5*colsum(x) for chunk c's tokens, replicated over the 128
    # partitions (fp32; the rank-1 mean-shift correction for fp8s mode)
    cq = (
        nc.dram_tensor("cq", [nch, 128, CS], f32, kind="ExternalInput").ap()
        if shifted
        else None
    )
    yT = nc.dram_tensor("yT", [D, Tp], f32, kind="ExternalOutput").ap()

    def mm_group(ps, tsz, nk, lhs_of, rhs_of, use_dr):
        """Accumulate nk k-tiles into psum ps[:, :tsz]; DoubleRow fuses
        pairs of k-tiles per matmul via 3D APs."""
        if use_dr:
            for j in range(0, nk, 2):
                nc.tensor.matmul(
                    ps[:, :tsz],
                    lhs_of(j, 2),
                    rhs_of(j, 2),
                    start=(j == 0),
                    stop=(j == nk - 2),
                    perf_mode=dr,
                )
        else:
            for j in range(nk):
                nc.tensor.matmul(
                    ps[:, :tsz],
                    lhs_of(j, 1),
                    rhs_of(j, 1),
                    start=(j == 0),
                    stop=(j == nk - 1),
                )

    with tile.TileContext(nc) as tc:
        with (
            tc.tile_pool(name="const", bufs=1) as const_pool,
            tc.tile_pool(name="acts", bufs=1) as acts_pool,
            tc.tile_pool(name="xtp", bufs=3) as xt_pool,
            tc.tile_pool(name="w1p", bufs=4) as w1_pool,
            tc.tile_pool(name="w2p", bufs=2) as w2_pool,
            tc.tile_pool(name="outp", bufs=4) as out_pool,
            tc.tile_pool(name="psum", bufs=7, space="PSUM") as psum_pool,
            tc.tile_pool(name="warm", bufs=1, space="PSUM") as warm_pool,
        ):
            b1_sb = const_pool.tile([128, HB], f32)
            b2_sb = const_pool.tile([128, DB], f32)

            # HAM warmup: ~8 dummy matmuls fill the PE while the first
            # DMAs land, so real matmuls start at 2.4 GHz instead of 1.2
            warm_sb = const_pool.tile([128, MM_N], l1_dt)
            nc.vector.memset(warm_sb[:], 0.0)
            warm_ps = warm_pool.tile([128, MM_N], f32, tag="warm")
            for _ in range(4):
                nc.tensor.matmul(
                    warm_ps[:, :MM_N],
                    warm_sb[:, :128],
                    warm_sb[:, :MM_N],
                    start=True,
                    stop=True,
                )

            for sup0 in range(0, nch, SUP // CS):

                cix = list(range(sup0, min(sup0 + SUP // CS, nch)))
                loffs = [offs[c] - offs[cix[0]] for c in cix]  # ht-local
                sup_len = sum(sizes[c] for c in cix)
                ht_sb = acts_pool.tile([128, HB, sup_len], l2_dt, tag="ht")

                # token chunks: chunk 0 split per k-pair on the sync ring
                # (its first pair gates the first matmul), the rest whole
                # on the gpsimd ring; w1 rides the sync ring after them
                xts = []
                cqs = []
                for ci, c in enumerate(cix):
                    xt_c = xt_pool.tile(
                        [128, KB_D, CS], l1_dt, tag=f"xt{ci}", bufs=1
                    )
                    xr = xq[c].rearrange("p (k m) -> p k m", k=KB_D)
                    if ci == 0 and sup0 == 0:
                        # two pieces: the first (k-pairs 0-1) gates the
                        # very first matmuls; each dma_start costs
                        # ~650 ns of issue time, so don't over-split
                        h_k = KB_D // 2
                        nc.sync.dma_start(
                            xt_c[:, :h_k, :], xr[:, :h_k, :]
                        )
                        nc.sync.dma_start(
                            xt_c[:, h_k:, :], xr[:, h_k:, :]
                        )
                    else:
                        nc.gpsimd.dma_start(xt_c[:], xr)
                    xts.append(xt_c)
                    if shifted:
                        cq_c = xt_pool.tile(
                            [128, CS], f32, tag=f"cq{ci}", bufs=1
                        )
                        cqs.append(cq_c)
                # cq after all xt chunks: the adds run well after the DMAs
                for ci, c in enumerate(cix):
                    if shifted:
                        (nc.sync if ci == 0 else nc.gpsimd).dma_start(
                            cqs[ci][:], cq[c]
                        )
                if sup0 == 0:
                    nc.sync.dma_start(b1_sb[:], b1[:])
                    nc.sync.dma_start(b2_sb[:], b2[:])

                # ---- layer 1: hT[h, c] ----
                for h in range(HB):
                    w1t = w1_pool.tile([128, KB_D, 128], l1_dt, tag="w1t")
                    w1r = w1[h].rearrange("p (k m) -> p k m", k=KB_D)
                    if h == 0 and sup0 == 0:
                        # first tile in two pieces on the idle scalar
                        # ring so its first half lands with xt0's
                        h_k = KB_D // 2
                        nc.scalar.dma_start(
                            w1t[:, :h_k, :], w1r[:, :h_k, :]
                        )
                        nc.scalar.dma_start(
                            w1t[:, h_k:, :], w1r[:, h_k:, :]
                        )
                    else:
                        nc.sync.dma_start(w1t[:], w1r)
                    for ci, c in enumerate(cix):
                        xt_c = xts[ci]
                        tsz = sizes[c]
                        lo = loffs[ci]
                        ps = psum_pool.tile([128, MM_N], f32, tag="ps")
                        mm_group(
                            ps,
                            tsz,
                            KB_D,
                            lambda j, w: w1t[:, j : j + w, :]
                            if w == 2
                            else w1t[:, j, :],
                            lambda j, w: xt_c[:, j : j + w, :tsz]
                            if w == 2
                            else xt_c[:, j, :tsz],
                            l1_dr,
                        )
                        if shifted:
                            # psum += c (per-token rank-1 mean correction)
                            nc.vector.scalar_tensor_tensor(
                                ps[:, :tsz],
                                ps[:, :tsz],
                                1.0,
                                cqs[ci][:, :tsz],
                                mybir.AluOpType.mult,
                                mybir.AluOpType.add,
                            )
                        nc.scalar.activation(
                            ht_sb[:, h, lo : lo + tsz],
                            ps[:, :tsz],
                            gelu,
                            bias=b1_sb[:, h : h + 1],
                        )

                # ---- layer 2: yT[d, c] ----
                for d in range(DB):
                    # w2 on the gpsimd (SWDGE) ring: parallel to the w1
                    # stream on the scalar ring, so d=0 prefetches early
                    w2t = w2_pool.tile([128, HB, 128], l2_dt, tag="w2t")
                    nc.gpsimd.dma_start(
                        w2t[:], w2[d].rearrange("p (k m) -> p k m", k=HB)
                    )
                    for ci, c in enumerate(cix):
                        tsz = sizes[c]
                        lo = loffs[ci]
                        go = offs[c]
                        ps = psum_pool.tile([128, MM_N], f32, tag="ps")
                        mm_group(
                            ps,
                            tsz,
                            HB,
                            lambda j, w: w2t[:, j : j + w, :]
                            if w == 2
                            else w2t[:, j, :],
                            lambda j, w: ht_sb[:, j : j + w, lo : lo + tsz]
                            if w == 2
                            else ht_sb[:, j, lo : lo + tsz],
                            l2_dr,
                        )
                        ot = out_pool.tile([128, MM_N], f32, tag="ot")
                        rings = [nc.scalar, nc.sync, nc.gpsimd]
                        if d == DB - 1:
                            # last d-tile: two pieces per chunk, each on
                            # a different ring so issue+transfer overlap
                            # and the post-matmul tail stays short
                            half = tsz // 2
                            pieces = [
                                (0, half, rings[(2 * ci) % 3]),
                                (half, tsz - half, rings[(2 * ci + 1) % 3]),
                            ]
                        else:
                            # bulk stores alternate sync/gpsimd (both
                            # idle during layer 2; scalar runs the ACTs)
                            eng = nc.sync if ci % 2 == 0 else nc.gpsimd
                            pieces = [(0, tsz, eng)]
                        for p0, psz, st_eng in pieces:
                            nc.scalar.activation(
                                ot[:, p0 : p0 + psz],
                                ps[:, p0 : p0 + psz],
                                ident,
                                bias=b2_sb[:, d : d + 1],
                            )
                            st_eng.dma_start(
                                yT[
                                    d * 128 : (d + 1) * 128,
                                    go + p0 : go + p0 + psz,
                                ],
                                ot[:, p0 : p0 + psz],
                            )

    nc.compile()
    return nc


def kernel(x, indices_s, weight1, weight2, bias1, bias2):
    from concourse import mybir
    from concourse.bass_utils import run_bass_kernel_spmd

    x = np.asarray(x, dtype=np.float32)
    idx = np.asarray(indices_s).astype(np.int64).ravel()
    w1_full = np.asarray(weight1, dtype=np.float32)
    w2_full = np.asarray(weight2, dtype=np.float32)
    b1_full = np.asarray(bias1, dtype=np.float32)
    b2_full = np.asarray(bias2, dtype=np.float32)

    order = np.argsort(idx, kind="stable")
    counts = np.bincount(idx, minlength=E)
    starts = np.concatenate([[0], np.cumsum(counts)])
    # tokens live in the free dim everywhere, so no alignment is needed:
    # every core computes exactly max(counts) token columns
    Tp = max(128, int(counts.max()))
    sizes = _chunk_sizes(Tp)
    nch = len(sizes)
    offs = np.concatenate([[0], np.cumsum(sizes)])

    mode = MODE
    key = (Tp, mode)
    nc = _program_cache.get(key)
    if nc is None:
        nc = _build_program(Tp, mode)
        _program_cache[key] = nc

    fp8_np = mybir.dt.np(mybir.dt.float8e4)
    l1_np = fp8_np if mode in ("fp8", "fp8l1", "fp8s") else BF16
    l2_np = fp8_np if mode in ("fp8", "fp8s") else BF16
    shifted = mode == "fp8s"
    w1_shift = np.float32(0.5) if shifted else np.float32(0.0)

    in_maps = []
    for e in range(E):
        toks = order[starts[e] : starts[e + 1]]
        # slot-aligned image: chunk c's tokens at columns [c*CS, c*CS+sizes[c])
        xTs = np.zeros((D, nch * CS), dtype=np.float32)
        for c in range(nch):
            lo, hi = offs[c], min(offs[c + 1], counts[e])
            if hi > lo:
                xTs[:, c * CS : c * CS + (hi - lo)] = x[toks[lo:hi]].T
        # [D, nch*CS] -> [nch, 128, KB_D*CS] chunk-major SBUF image
        xq = (
            np.ascontiguousarray(
                xTs.reshape(KB_D, 128, nch, CS).transpose(2, 1, 0, 3)
            )
            .reshape(nch, 128, KB_D * CS)
            .astype(l1_np)
        )
        w1r = (
            np.ascontiguousarray(
                (w1_full[e] - w1_shift)
                .reshape(KB_D, 128, HB, 128)
                .transpose(2, 1, 0, 3)
            )
            .reshape(HB, 128, KB_D * 128)
            .astype(l1_np)
        )
        w2r = (
            np.ascontiguousarray(
                w2_full[e].reshape(HB, 128, DB, 128).transpose(2, 1, 0, 3)
            )
            .reshape(DB, 128, HB * 128)
            .astype(l2_np)
        )
        b1d = np.ascontiguousarray(b1_full[e].reshape(HB, 128).T)
        b2d = np.ascontiguousarray(b2_full[e].reshape(DB, 128).T)
        im = {"xq": xq, "w1": w1r, "w2": w2r, "b1": b1d, "b2": b2d}
        if shifted:
            # c[t] = 0.5 * sum_d x[t, d] in fp32, slot-aligned like xq,
            # replicated across the 128 partitions
            cvals = np.zeros((nch * CS,), dtype=np.float32)
            for c in range(nch):
                lo, hi = offs[c], min(offs[c + 1], counts[e])
                if hi > lo:
                    cvals[c * CS : c * CS + (hi - lo)] = (
                        0.5 * x[toks[lo:hi]].sum(axis=1)
                    )
            im["cq"] = np.ascontiguousarray(
                np.broadcast_to(
                    cvals.reshape(nch, 1, CS), (nch, 128, CS)
                )
            )
        in_maps.append(im)

    res = run_bass_kernel_spmd(
        nc,
        in_maps,
        list(range(N_CORES)),
        trace=os.environ.get("BASS_TRACE") == "1",
    )
    global last_results
    last_results = res

    out = np.empty((T, D), dtype=np.float32)
    for e in range(E):
        toks = order[starts[e] : starts[e + 1]]
        out[toks] = res.results[e]["yT"][:, : counts[e]].T
    if res.exec_time_ns is not None:
        print(f"HW exec time: {res.exec_time_ns} ns")
    return out[:, None, :]



# revision 21
# speedup vs baseline: 1.2654x; 1.0407x over previous
"""MoE runtime-experts kernel for 8 Trainium2 NeuronCores.

Problem: y[t] = gelu(x[t] @ W1[e] + b1[e]) @ W2[e] + b2[e], e = indices[t].
T=8192 tokens, D=1024, H=4096, E=8 experts.

Strategy: expert-parallel. Host routes tokens by expert (argsort), core e
gets expert e's weights plus its tokens (transposed, zero-padded to a
common Tp so all 8 cores run one SPMD program). On device each core runs a
dense 2-layer MLP with fp32 PSUM accumulation:

  layer 1: hT[h, t] = gelu(sum_d W1[d, h] * xT[d, t] + b1[h])
           (lhsT = W1 k-tile [128d, 128h], rhs = xT [128d, 384t])
  layer 2: yT[d, t] = sum_h W2[h, d] * hT[h, t] + b2[d]
           (lhsT = W2 h-tile [128h, 128d], rhs = hT [128h, 384t])

Both layers keep the token axis in the free dimension, so no on-device
transpose is needed anywhere — and because tokens are always a free dim,
Tp needs no alignment: every core computes exactly max(counts) token
columns, split into balanced chunks of <=384 (one fp32 PSUM bank each).
Token-chunk DMAs are spread across the sync and gpsimd rings while the
scalar ring streams w1, so the PE starts ~13 us in and stays >=90% busy.
Host un-permutes yT shards into the full [T, 1, D] output.

KERNEL_MODE selects compute dtype: "fp8s" (default; both layers fp8e4m3
+ DoubleRow, with W1 sent as W1-0.5 and the exact rank-1 correction
c[t] = 0.5*sum_d x[d,t] computed on host in fp32 and added on-device by
the vector engine before gelu — this removes the common-mode error of
naive fp8 that fails the 2e-2 gate), "bf16", "fp8" (naive fp8, fails
the gate), "fp8l1" (layer 1 fp8, layer 2 bf16).
"""

import math
import os

import numpy as np
import ml_dtypes

T, D, H, E = 8192, 1024, 4096, 8
N_CORES = 8
KB_D = D // 128  # 8  k-tiles of the D contraction
HB = H // 128  # 32 h-tiles
DB = D // 128  # 8  d-tiles
BF16 = ml_dtypes.bfloat16
CS = 384  # token chunk (matmul moving-operand free dim)
SUP = 4 * CS  # tokens resident per pass (SBUF limit)
MM_N = 512  # PSUM bank free size (fp32)

MODE = os.environ.get("KERNEL_MODE", "fp8s")

_program_cache: dict[tuple, object] = {}
last_results = None  # BassKernelResults of the most recent kernel() call


def _chunk_sizes(Tp: int):
    """Balanced split of Tp token columns into chunks of at most CS."""
    nch = max(1, math.ceil(Tp / CS))
    base, rem = divmod(Tp, nch)
    return [base + (1 if i < rem else 0) for i in range(nch)]


def _build_program(Tp: int, mode: str):
    import concourse.tile as tile
    from concourse import bacc, mybir

    sizes = _chunk_sizes(Tp)
    nch = len(sizes)
    offs = [sum(sizes[:i]) for i in range(nch)]  # global token offsets

    f32 = mybir.dt.float32
    bf16 = mybir.dt.bfloat16
    fp8 = mybir.dt.float8e4
    l1_dt = fp8 if mode in ("fp8", "fp8l1", "fp8s") else bf16
    l2_dt = fp8 if mode in ("fp8", "fp8s") else bf16
    shifted = mode == "fp8s"
    l1_dr = l1_dt == fp8
    l2_dr = l2_dt == fp8
    dr = mybir.MatmulPerfMode.DoubleRow
    gelu = mybir.ActivationFunctionType.Gelu
    ident = mybir.ActivationFunctionType.Identity

    nc = bacc.Bacc(
        "TRN2", target_bir_lowering=False, debug=False, num_devices=N_CORES
    )

    # xq[c] is the SBUF image of token chunk c: [128, KB_D*CS], row-major
    # (kb, t) per partition, so the DMA is fully contiguous
    xq = nc.dram_tensor(
        "xq", [nch, 128, KB_D * CS], l1_dt, kind="ExternalInput"
    ).ap()
    # w1[h] is a [128, KB_D*128] block: col-chunk kb holds W1[kb*128+p, h*128+m]
    w1 = nc.dram_tensor(
        "w1", [HB, 128, KB_D * 128], l1_dt, kind="ExternalInput"
    ).ap()
    # w2[d] is a [128, HB*128] block: col-chunk hb holds W2[hb*128+p, d*128+m]
    w2 = nc.dram_tensor(
        "w2", [DB, 128, HB * 128], l2_dt, kind="ExternalInput"
    ).ap()
    b1 = nc.dram_tensor("b1", [128, HB], f32, kind="ExternalInput").ap()
    b2 = nc.dram_tensor("b2", [128, DB], f32, kind="ExternalInput").ap()
    # cq[c] = 0.5*colsum(x) for chunk c's tokens, replicated over the 128
    # partitions (fp32; the rank-1 mean-shift correction for fp8s mode)
    cq = (
        nc.dram_tensor("cq", [nch, 128, CS], f32, kind="ExternalInput").ap()
        if shifted
        else None
    )
    yT = nc.dram_tensor("yT", [D, Tp], f32, kind="ExternalOutput").ap()

    def mm_group(ps, tsz, nk, lhs_of, rhs_of, use_dr):
        """Accumulate nk k-tiles into psum ps[:, :tsz]; DoubleRow fuses
        pairs of k-tiles per matmul via 3D APs."""
        if use_dr:
            for j in range(0, nk, 2):
                nc.tensor.matmul(
                    ps[:, :tsz],
                    lhs_of(j, 2),
                    rhs_of(j, 2),
                    start=(j == 0),
                    stop=(j == nk - 2),
                    perf_mode=dr,
                )
        else:
            for j in range(nk):
                nc.tensor.matmul(
                    ps[:, :tsz],
                    lhs_of(j, 1),
                    rhs_of(j, 1),
                    start=(j == 0),
                    stop=(j == nk - 1),
                )

    with tile.TileContext(nc) as tc:
        with (
            tc.tile_pool(name="const", bufs=1) as const_pool,
            tc.tile_pool(name="acts", bufs=1) as acts_pool,
            tc.tile_pool(name="xtp", bufs=3) as xt_pool,
            tc.tile_pool(name="w1p", bufs=4) as w1_pool,
            tc.tile_pool(name="w2p", bufs=2) as w2_pool,
            tc.tile_pool(name="outp", bufs=4) as out_pool,
            tc.tile_pool(name="psum", bufs=7, space="PSUM") as psum_pool,
            tc.tile_pool(name="warm", bufs=1, space="PSUM") as warm_pool,
        ):
            b1_sb = const_pool.tile([128, HB], f32)
            b2_sb = const_pool.tile([128, DB], f32)

            # HAM warmup: ~8 dummy matmuls fill the PE while the first
            # DMAs land, so real matmuls start at 2.4 GHz instead of 1.2
            warm_sb = const_pool.tile([128, MM_N], l1_dt)
            nc.vector.memset(warm_sb[:], 0.0)
            warm_ps = warm_pool.tile([128, MM_N], f32, tag="warm")
            for _ in range(4):
                nc.tensor.matmul(
                    warm_ps[:, :MM_N],
                    warm_sb[:, :128],
                    warm_sb[:, :MM_N],
                    start=True,
                    stop=True,
                )

            for sup0 in range(0, nch, SUP // CS):

                cix = list(range(sup0, min(sup0 + SUP // CS, nch)))
                loffs = [offs[c] - offs[cix[0]] for c in cix]  # ht-local
                sup_len = sum(sizes[c] for c in cix)
                ht_sb = acts_pool.tile([128, HB, sup_len], l2_dt, tag="ht")

                # token chunks: chunk 0 split per k-pair on the sync ring
                # (its first pair gates the first matmul), the rest whole
                # on the gpsimd ring; w1 rides the sync ring after them
                # DMA rings serialize transfers end-to-end (~2-2.5 us
                # each), so the early queues are ordered by when each
                # tile is first needed: sync=[xt0, w1 stream],
                # gpsimd=[xt1, xt2, cq1, cq2, w2...],
                # scalar=[w1[0], b1, cq0, b2] (idle until first gelu)
                xts = []
                cqs = []
                for ci, c in enumerate(cix):
                    xt_c = xt_pool.tile(
                        [128, KB_D, CS], l1_dt, tag=f"xt{ci}", bufs=1
                    )
                    xr = xq[c].rearrange("p (k m) -> p k m", k=KB_D)
                    (nc.sync if ci == 0 else nc.gpsimd).dma_start(
                        xt_c[:], xr
                    )
                    xts.append(xt_c)
                    if shifted:
                        cq_c = xt_pool.tile(
                            [128, CS], f32, tag=f"cq{ci}", bufs=1
                        )
                        cqs.append(cq_c)
                if shifted:
                    for ci, c in enumerate(cix):
                        # chunk 0 of the first pass loads on the scalar
                        # ring inside the ramp block below
                        if ci > 0 or sup0 > 0:
                            nc.gpsimd.dma_start(cqs[ci][:], cq[c])

                # ---- layer 1: hT[h, c] ----
                def w1_load(h):
                    w1t = w1_pool.tile(
                        [128, KB_D, 128], l1_dt, tag="w1t"
                    )
                    w1r = w1[h].rearrange("p (k m) -> p k m", k=KB_D)
                    eng = (
                        nc.scalar if h == 0 and sup0 == 0 else nc.sync
                    )
                    eng.dma_start(w1t[:], w1r)
                    return w1t

                def l1_group(h, w1t, ci):
                    c = cix[ci]
                    xt_c = xts[ci]
                    tsz = sizes[c]
                    lo = loffs[ci]
                    ps = psum_pool.tile([128, MM_N], f32, tag="ps")
                    mm_group(
                        ps,
                        tsz,
                        KB_D,
                        lambda j, w: w1t[:, j : j + w, :]
                        if w == 2
                        else w1t[:, j, :],
                        lambda j, w: xt_c[:, j : j + w, :tsz]
                        if w == 2
                        else xt_c[:, j, :tsz],
                        l1_dr,
                    )
                    if shifted:
                        # psum += c (per-token rank-1 mean correction)
                        nc.vector.scalar_tensor_tensor(
                            ps[:, :tsz],
                            ps[:, :tsz],
                            1.0,
                            cqs[ci][:, :tsz],
                            mybir.AluOpType.mult,
                            mybir.AluOpType.add,
                        )
                    nc.scalar.activation(
                        ht_sb[:, h, lo : lo + tsz],
                        ps[:, :tsz],
                        gelu,
                        bias=b1_sb[:, h : h + 1],
                    )

                if sup0 == 0:
                    # ramp: first two h-tiles chunk-major, matching DMA
                    # arrival order (xt0 first, then xt1, xt2), so the
                    # PE never waits on a late chunk during warm-up
                    w1t0 = w1_load(0)
                    nc.scalar.dma_start(b1_sb[:], b1[:])
                    if shifted:
                        nc.scalar.dma_start(cqs[0][:], cq[cix[0]])
                    nc.scalar.dma_start(b2_sb[:], b2[:])
                    w1t1 = w1_load(1)
                    for ci in range(len(cix)):
                        l1_group(0, w1t0, ci)
                        l1_group(1, w1t1, ci)
                    h_start = 2
                else:
                    h_start = 0
                for h in range(h_start, HB):
                    w1t = w1_load(h)
                    for ci in range(len(cix)):
                        l1_group(h, w1t, ci)

                # ---- layer 2: yT[d, c] ----
                for d in range(DB):
                    # w2 on the gpsimd (SWDGE) ring: parallel to the w1
                    # stream on the scalar ring, so d=0 prefetches early
                    w2t = w2_pool.tile([128, HB, 128], l2_dt, tag="w2t")
                    nc.gpsimd.dma_start(
                        w2t[:], w2[d].rearrange("p (k m) -> p k m", k=HB)
                    )
                    for ci, c in enumerate(cix):
                        tsz = sizes[c]
                        lo = loffs[ci]
                        go = offs[c]
                        ps = psum_pool.tile([128, MM_N], f32, tag="ps")
                        mm_group(
                            ps,
                            tsz,
                            HB,
                            lambda j, w: w2t[:, j : j + w, :]
                            if w == 2
                            else w2t[:, j, :],
                            lambda j, w: ht_sb[:, j : j + w, lo : lo + tsz]
                            if w == 2
                            else ht_sb[:, j, lo : lo + tsz],
                            l2_dr,
                        )
                        ot = out_pool.tile([128, MM_N], f32, tag="ot")
                        rings = [nc.scalar, nc.sync, nc.gpsimd]
                        if d == DB - 1:
                            # last d-tile: two pieces per chunk, each on
                            # a different ring so issue+transfer overlap
                            # and the post-matmul tail stays short
                            half = tsz // 2
                            pieces = [
                                (0, half, rings[(2 * ci) % 3]),
                                (half, tsz - half, rings[(2 * ci + 1) % 3]),
                            ]
                        else:
                            # bulk stores alternate sync/gpsimd (both
                            # idle during layer 2; scalar runs the ACTs)
                            eng = nc.sync if ci % 2 == 0 else nc.gpsimd
                            pieces = [(0, tsz, eng)]
                        for p0, psz, st_eng in pieces:
                            nc.scalar.activation(
                                ot[:, p0 : p0 + psz],
                                ps[:, p0 : p0 + psz],
                                ident,
                                bias=b2_sb[:, d : d + 1],
                            )
                            st_eng.dma_start(
                                yT[
                                    d * 128 : (d + 1) * 128,
                                    go + p0 : go + p0 + psz,
                                ],
                                ot[:, p0 : p0 + psz],
                            )

    nc.compile()
    return nc


def kernel(x, indices_s, weight1, weight2, bias1, bias2):
    from concourse import mybir
    from concourse.bass_utils import run_bass_kernel_spmd

    x = np.asarray(x, dtype=np.float32)
    idx = np.asarray(indices_s).astype(np.int64).ravel()
    w1_full = np.asarray(weight1, dtype=np.float32)
    w2_full = np.asarray(weight2, dtype=np.float32)
    b1_full = np.asarray(bias1, dtype=np.float32)
    b2_full = np.asarray(bias2, dtype=np.float32)

    order = np.argsort(idx, kind="stable")
    counts = np.bincount(idx, minlength=E)
    starts = np.concatenate([[0], np.cumsum(counts)])
    # tokens live in the free dim everywhere, so no alignment is needed:
    # every core computes exactly max(counts) token columns
    Tp = max(128, int(counts.max()))
    sizes = _chunk_sizes(Tp)
    nch = len(sizes)
    offs = np.concatenate([[0], np.cumsum(sizes)])

    mode = MODE
    key = (Tp, mode)
    nc = _program_cache.get(key)
    if nc is None:
        nc = _build_program(Tp, mode)
        _program_cache[key] = nc

    fp8_np = mybir.dt.np(mybir.dt.float8e4)
    l1_np = fp8_np if mode in ("fp8", "fp8l1", "fp8s") else BF16
    l2_np = fp8_np if mode in ("fp8", "fp8s") else BF16
    shifted = mode == "fp8s"
    w1_shift = np.float32(0.5) if shifted else np.float32(0.0)

    in_maps = []
    for e in range(E):
        toks = order[starts[e] : starts[e + 1]]
        # slot-aligned image: chunk c's tokens at columns [c*CS, c*CS+sizes[c])
        xTs = np.zeros((D, nch * CS), dtype=np.float32)
        for c in range(nch):
            lo, hi = offs[c], min(offs[c + 1], counts[e])
            if hi > lo:
                xTs[:, c * CS : c * CS + (hi - lo)] = x[toks[lo:hi]].T
        # [D, nch*CS] -> [nch, 128, KB_D*CS] chunk-major SBUF image
        xq = (
            np.ascontiguousarray(
                xTs.reshape(KB_D, 128, nch, CS).transpose(2, 1, 0, 3)
            )
            .reshape(nch, 128, KB_D * CS)
            .astype(l1_np)
        )
        w1r = (
            np.ascontiguousarray(
                (w1_full[e] - w1_shift)
                .reshape(KB_D, 128, HB, 128)
                .transpose(2, 1, 0, 3)
            )
            .reshape(HB, 128, KB_D * 128)
            .astype(l1_np)
        )
        w2r = (
            np.ascontiguousarray(
                w2_full[e].reshape(HB, 128, DB, 128).transpose(2, 1, 0, 3)
            )
            .reshape(DB, 128, HB * 128)
            .astype(l2_np)
        )
        b1d = np.ascontiguousarray(b1_full[e].reshape(HB, 128).T)
        b2d = np.ascontiguousarray(b2_full[e].reshape(DB, 128).T)
        im = {"xq": xq, "w1": w1r, "w2": w2r, "b1": b1d, "b2": b2d}
        if shifted:
            # c[t] = 0.5 * sum_d x[t, d] in fp32, slot-aligned like xq,
            # replicated across the 128 partitions
            cvals = np.zeros((nch * CS,), dtype=np.float32)
            for c in range(nch):
                lo, hi = offs[c], min(offs[c + 1], counts[e])
                if hi > lo:
                    cvals[c * CS : c * CS + (hi - lo)] = (
                        0.5 * x[toks[lo:hi]].sum(axis=1)
                    )
            im["cq"] = np.ascontiguousarray(
                np.broadcast_to(
                    cvals.reshape(nch, 1, CS), (nch, 128, CS)
                )
            )
        in_maps.append(im)

    res = run_bass_kernel_spmd(
        nc,
        in_maps,
        list(range(N_CORES)),
        trace=os.environ.get("BASS_TRACE") == "1",
    )
    global last_results
    last_results = res

    out = np.empty((T, D), dtype=np.float32)
    for e in range(E):
        toks = order[starts[e] : starts[e + 1]]
        out[toks] = res.results[e]["yT"][:, : counts[e]].T
    if res.exec_time_ns is not None:
        print(f"HW exec time: {res.exec_time_ns} ns")
    return out[:, None, :]



# revision 23
# speedup vs baseline: 1.2674x; 1.0016x over previous
"""MoE runtime-experts kernel for 8 Trainium2 NeuronCores.

Problem: y[t] = gelu(x[t] @ W1[e] + b1[e]) @ W2[e] + b2[e], e = indices[t].
T=8192 tokens, D=1024, H=4096, E=8 experts.

Strategy: expert-parallel. Host routes tokens by expert (argsort), core e
gets expert e's weights plus its tokens (transposed, zero-padded to a
common Tp so all 8 cores run one SPMD program). On device each core runs a
dense 2-layer MLP with fp32 PSUM accumulation:

  layer 1: hT[h, t] = gelu(sum_d W1[d, h] * xT[d, t] + b1[h])
           (lhsT = W1 k-tile [128d, 128h], rhs = xT [128d, 384t])
  layer 2: yT[d, t] = sum_h W2[h, d] * hT[h, t] + b2[d]
           (lhsT = W2 h-tile [128h, 128d], rhs = hT [128h, 384t])

Both layers keep the token axis in the free dimension, so no on-device
transpose is needed anywhere — and because tokens are always a free dim,
Tp needs no alignment: every core computes exactly max(counts) token
columns, split into balanced chunks of <=384 (one fp32 PSUM bank each).
Token-chunk DMAs are spread across the sync and gpsimd rings while the
scalar ring streams w1, so the PE starts ~13 us in and stays >=90% busy.
Host un-permutes yT shards into the full [T, 1, D] output.

KERNEL_MODE selects compute dtype: "fp8s" (default; both layers fp8e4m3
+ DoubleRow, with W1 sent as W1-0.5 and the exact rank-1 correction
c[t] = 0.5*sum_d x[d,t] computed on host in fp32 and added on-device by
the vector engine before gelu — this removes the common-mode error of
naive fp8 that fails the 2e-2 gate), "bf16", "fp8" (naive fp8, fails
the gate), "fp8l1" (layer 1 fp8, layer 2 bf16).
"""

import math
import os

import numpy as np
import ml_dtypes

T, D, H, E = 8192, 1024, 4096, 8
N_CORES = 8
KB_D = D // 128  # 8  k-tiles of the D contraction
HB = H // 128  # 32 h-tiles
DB = D // 128  # 8  d-tiles
BF16 = ml_dtypes.bfloat16
CS = 384  # token chunk (matmul moving-operand free dim)
SUP = 4 * CS  # tokens resident per pass (SBUF limit)
MM_N = 512  # PSUM bank free size (fp32)

MODE = os.environ.get("KERNEL_MODE", "fp8s")

_program_cache: dict[tuple, object] = {}
last_results = None  # BassKernelResults of the most recent kernel() call


def _chunk_sizes(Tp: int):
    """Balanced split of Tp token columns into chunks of at most CS."""
    nch = max(1, math.ceil(Tp / CS))
    base, rem = divmod(Tp, nch)
    return [base + (1 if i < rem else 0) for i in range(nch)]


def _build_program(Tp: int, mode: str):
    import concourse.tile as tile
    from concourse import bacc, mybir

    sizes = _chunk_sizes(Tp)
    nch = len(sizes)
    offs = [sum(sizes[:i]) for i in range(nch)]  # global token offsets

    f32 = mybir.dt.float32
    bf16 = mybir.dt.bfloat16
    fp8 = mybir.dt.float8e4
    l1_dt = fp8 if mode in ("fp8", "fp8l1", "fp8s") else bf16
    l2_dt = fp8 if mode in ("fp8", "fp8s") else bf16
    shifted = mode == "fp8s"
    l1_dr = l1_dt == fp8
    l2_dr = l2_dt == fp8
    dr = mybir.MatmulPerfMode.DoubleRow
    gelu = mybir.ActivationFunctionType.Gelu
    ident = mybir.ActivationFunctionType.Identity

    nc = bacc.Bacc(
        "TRN2", target_bir_lowering=False, debug=False, num_devices=N_CORES
    )

    # xq[c] is the SBUF image of token chunk c: [128, KB_D*CS], row-major
    # (kb, t) per partition, so the DMA is fully contiguous
    xq = nc.dram_tensor(
        "xq", [nch, 128, KB_D * CS], l1_dt, kind="ExternalInput"
    ).ap()
    # w1[h] is a [128, KB_D*128] block: col-chunk kb holds W1[kb*128+p, h*128+m]
    w1 = nc.dram_tensor(
        "w1", [HB, 128, KB_D * 128], l1_dt, kind="ExternalInput"
    ).ap()
    # w2[d] is a [128, HB*128] block: col-chunk hb holds W2[hb*128+p, d*128+m]
    w2 = nc.dram_tensor(
        "w2", [DB, 128, HB * 128], l2_dt, kind="ExternalInput"
    ).ap()
    b1 = nc.dram_tensor("b1", [128, HB], f32, kind="ExternalInput").ap()
    b2 = nc.dram_tensor("b2", [128, DB], f32, kind="ExternalInput").ap()
    # cq[c] = 0.5*colsum(x) for chunk c's tokens, replicated over the 128
    # partitions (fp32; the rank-1 mean-shift correction for fp8s mode)
    cq = (
        nc.dram_tensor("cq", [nch, 128, CS], f32, kind="ExternalInput").ap()
        if shifted
        else None
    )
    yT = nc.dram_tensor("yT", [D, Tp], f32, kind="ExternalOutput").ap()

    def mm_group(ps, tsz, nk, lhs_of, rhs_of, use_dr):
        """Accumulate nk k-tiles into psum ps[:, :tsz]; DoubleRow fuses
        pairs of k-tiles per matmul via 3D APs."""
        if use_dr:
            for j in range(0, nk, 2):
                nc.tensor.matmul(
                    ps[:, :tsz],
                    lhs_of(j, 2),
                    rhs_of(j, 2),
                    start=(j == 0),
                    stop=(j == nk - 2),
                    perf_mode=dr,
                )
        else:
            for j in range(nk):
                nc.tensor.matmul(
                    ps[:, :tsz],
                    lhs_of(j, 1),
                    rhs_of(j, 1),
                    start=(j == 0),
                    stop=(j == nk - 1),
                )

    with tile.TileContext(nc) as tc:
        with (
            tc.tile_pool(name="const", bufs=1) as const_pool,
            tc.tile_pool(name="acts", bufs=1) as acts_pool,
            tc.tile_pool(name="xtp", bufs=3) as xt_pool,
            tc.tile_pool(name="w1p", bufs=4) as w1_pool,
            tc.tile_pool(name="w2p", bufs=2) as w2_pool,
            tc.tile_pool(name="outp", bufs=4) as out_pool,
            tc.tile_pool(name="psum", bufs=7, space="PSUM") as psum_pool,
            tc.tile_pool(name="warm", bufs=1, space="PSUM") as warm_pool,
        ):
            b1_sb = const_pool.tile([128, HB], f32)
            b2_sb = const_pool.tile([128, DB], f32)

            # HAM warmup: ~8 dummy matmuls fill the PE while the first
            # DMAs land, so real matmuls start at 2.4 GHz instead of 1.2
            warm_sb = const_pool.tile([128, MM_N], l1_dt)
            nc.vector.memset(warm_sb[:], 0.0)
            warm_ps = warm_pool.tile([128, MM_N], f32, tag="warm")
            for _ in range(7):
                nc.tensor.matmul(
                    warm_ps[:, :MM_N],
                    warm_sb[:, :128],
                    warm_sb[:, :MM_N],
                    start=True,
                    stop=True,
                )

            for sup0 in range(0, nch, SUP // CS):

                cix = list(range(sup0, min(sup0 + SUP // CS, nch)))
                loffs = [offs[c] - offs[cix[0]] for c in cix]  # ht-local
                sup_len = sum(sizes[c] for c in cix)
                ht_sb = acts_pool.tile([128, HB, sup_len], l2_dt, tag="ht")

                # token chunks: chunk 0 split per k-pair on the sync ring
                # (its first pair gates the first matmul), the rest whole
                # on the gpsimd ring; w1 rides the sync ring after them
                # DMA rings serialize transfers end-to-end (~2-2.5 us
                # each), so the early queues are ordered by when each
                # tile is first needed: sync=[xt0, w1 stream],
                # gpsimd=[xt1, xt2, cq1, cq2, w2...],
                # scalar=[w1[0], b1, cq0, b2] (idle until first gelu)
                xts = []
                cqs = []
                for ci, c in enumerate(cix):
                    xt_c = xt_pool.tile(
                        [128, KB_D, CS], l1_dt, tag=f"xt{ci}", bufs=1
                    )
                    xr = xq[c].rearrange("p (k m) -> p k m", k=KB_D)
                    (nc.sync if ci == 0 else nc.gpsimd).dma_start(
                        xt_c[:], xr
                    )
                    xts.append(xt_c)
                    if shifted:
                        cq_c = xt_pool.tile(
                            [128, CS], f32, tag=f"cq{ci}", bufs=1
                        )
                        cqs.append(cq_c)
                if shifted:
                    for ci, c in enumerate(cix):
                        # chunk 0 of the first pass loads on the scalar
                        # ring inside the ramp block below
                        if ci > 0 or sup0 > 0:
                            nc.gpsimd.dma_start(cqs[ci][:], cq[c])

                # ---- layer 1: hT[h, c] ----
                def w1_load(h):
                    w1t = w1_pool.tile(
                        [128, KB_D, 128], l1_dt, tag="w1t"
                    )
                    w1r = w1[h].rearrange("p (k m) -> p k m", k=KB_D)
                    eng = (
                        nc.scalar if h == 0 and sup0 == 0 else nc.sync
                    )
                    eng.dma_start(w1t[:], w1r)
                    return w1t

                def l1_group(h, w1t, ci):
                    c = cix[ci]
                    xt_c = xts[ci]
                    tsz = sizes[c]
                    lo = loffs[ci]
                    ps = psum_pool.tile([128, MM_N], f32, tag="ps")
                    mm_group(
                        ps,
                        tsz,
                        KB_D,
                        lambda j, w: w1t[:, j : j + w, :]
                        if w == 2
                        else w1t[:, j, :],
                        lambda j, w: xt_c[:, j : j + w, :tsz]
                        if w == 2
                        else xt_c[:, j, :tsz],
                        l1_dr,
                    )
                    if shifted:
                        # psum += c (per-token rank-1 mean correction)
                        nc.vector.scalar_tensor_tensor(
                            ps[:, :tsz],
                            ps[:, :tsz],
                            1.0,
                            cqs[ci][:, :tsz],
                            mybir.AluOpType.mult,
                            mybir.AluOpType.add,
                        )
                    nc.scalar.activation(
                        ht_sb[:, h, lo : lo + tsz],
                        ps[:, :tsz],
                        gelu,
                        bias=b1_sb[:, h : h + 1],
                    )

                if sup0 == 0:
                    # ramp: first two h-tiles chunk-major, matching DMA
                    # arrival order (xt0 first, then xt1, xt2), so the
                    # PE never waits on a late chunk during warm-up
                    w1t0 = w1_load(0)
                    nc.scalar.dma_start(b1_sb[:], b1[:])
                    if shifted:
                        nc.scalar.dma_start(cqs[0][:], cq[cix[0]])
                    nc.scalar.dma_start(b2_sb[:], b2[:])
                    w1t1 = w1_load(1)
                    for ci in range(len(cix)):
                        l1_group(0, w1t0, ci)
                        l1_group(1, w1t1, ci)
                    h_start = 2
                else:
                    h_start = 0
                for h in range(h_start, HB):
                    w1t = w1_load(h)
                    for ci in range(len(cix)):
                        l1_group(h, w1t, ci)

                # ---- layer 2: yT[d, c] ----
                for d in range(DB):
                    # w2 on the gpsimd (SWDGE) ring: parallel to the w1
                    # stream on the scalar ring, so d=0 prefetches early
                    w2t = w2_pool.tile([128, HB, 128], l2_dt, tag="w2t")
                    nc.gpsimd.dma_start(
                        w2t[:], w2[d].rearrange("p (k m) -> p k m", k=HB)
                    )
                    for ci, c in enumerate(cix):
                        tsz = sizes[c]
                        lo = loffs[ci]
                        go = offs[c]
                        ps = psum_pool.tile([128, MM_N], f32, tag="ps")
                        mm_group(
                            ps,
                            tsz,
                            HB,
                            lambda j, w: w2t[:, j : j + w, :]
                            if w == 2
                            else w2t[:, j, :],
                            lambda j, w: ht_sb[:, j : j + w, lo : lo + tsz]
                            if w == 2
                            else ht_sb[:, j, lo : lo + tsz],
                            l2_dr,
                        )
                        ot = out_pool.tile([128, MM_N], f32, tag="ot")
                        rings = [nc.scalar, nc.sync, nc.gpsimd]
                        if d == DB - 1:
                            # last d-tile: two pieces per chunk, each on
                            # a different ring so issue+transfer overlap
                            # and the post-matmul tail stays short
                            cut = tsz - 128
                            pieces = [
                                (0, cut, rings[(2 * ci) % 3]),
                                (cut, tsz - cut, rings[(2 * ci + 1) % 3]),
                            ]
                        else:
                            # bulk stores alternate sync/gpsimd (both
                            # idle during layer 2; scalar runs the ACTs)
                            eng = nc.sync if ci % 2 == 0 else nc.gpsimd
                            pieces = [(0, tsz, eng)]
                        for p0, psz, st_eng in pieces:
                            nc.scalar.activation(
                                ot[:, p0 : p0 + psz],
                                ps[:, p0 : p0 + psz],
                                ident,
                                bias=b2_sb[:, d : d + 1],
                            )
                            st_eng.dma_start(
                                yT[
                                    d * 128 : (d + 1) * 128,
                                    go + p0 : go + p0 + psz,
                                ],
                                ot[:, p0 : p0 + psz],
                            )

    nc.compile()
    return nc


def kernel(x, indices_s, weight1, weight2, bias1, bias2):
    from concourse import mybir
    from concourse.bass_utils import run_bass_kernel_spmd

    x = np.asarray(x, dtype=np.float32)
    idx = np.asarray(indices_s).astype(np.int64).ravel()
    w1_full = np.asarray(weight1, dtype=np.float32)
    w2_full = np.asarray(weight2, dtype=np.float32)
    b1_full = np.asarray(bias1, dtype=np.float32)
    b2_full = np.asarray(bias2, dtype=np.float32)

    order = np.argsort(idx, kind="stable")
    counts = np.bincount(idx, minlength=E)
    starts = np.concatenate([[0], np.cumsum(counts)])
    # tokens live in the free dim everywhere, so no alignment is needed:
    # every core computes exactly max(counts) token columns
    Tp = max(128, int(counts.max()))
    sizes = _chunk_sizes(Tp)
    nch = len(sizes)
    offs = np.concatenate([[0], np.cumsum(sizes)])

    mode = MODE
    key = (Tp, mode)
    nc = _program_cache.get(key)
    if nc is None:
        nc = _build_program(Tp, mode)
        _program_cache[key] = nc

    fp8_np = mybir.dt.np(mybir.dt.float8e4)
    l1_np = fp8_np if mode in ("fp8", "fp8l1", "fp8s") else BF16
    l2_np = fp8_np if mode in ("fp8", "fp8s") else BF16
    shifted = mode == "fp8s"
    w1_shift = np.float32(0.5) if shifted else np.float32(0.0)

    in_maps = []
    for e in range(E):
        toks = order[starts[e] : starts[e + 1]]
        # slot-aligned image: chunk c's tokens at columns [c*CS, c*CS+sizes[c])
        xTs = np.zeros((D, nch * CS), dtype=np.float32)
        for c in range(nch):
            lo, hi = offs[c], min(offs[c + 1], counts[e])
            if hi > lo:
                xTs[:, c * CS : c * CS + (hi - lo)] = x[toks[lo:hi]].T
        # [D, nch*CS] -> [nch, 128, KB_D*CS] chunk-major SBUF image
        xq = (
            np.ascontiguousarray(
                xTs.reshape(KB_D, 128, nch, CS).transpose(2, 1, 0, 3)
            )
            .reshape(nch, 128, KB_D * CS)
            .astype(l1_np)
        )
        w1r = (
            np.ascontiguousarray(
                (w1_full[e] - w1_shift)
                .reshape(KB_D, 128, HB, 128)
                .transpose(2, 1, 0, 3)
            )
            .reshape(HB, 128, KB_D * 128)
            .astype(l1_np)
        )
        w2r = (
            np.ascontiguousarray(
                w2_full[e].reshape(HB, 128, DB, 128).transpose(2, 1, 0, 3)
            )
            .reshape(DB, 128, HB * 128)
            .astype(l2_np)
        )
        b1d = np.ascontiguousarray(b1_full[e].reshape(HB, 128).T)
        b2d = np.ascontiguousarray(b2_full[e].reshape(DB, 128).T)
        im = {"xq": xq, "w1": w1r, "w2": w2r, "b1": b1d, "b2": b2d}
        if shifted:
            # c[t] = 0.5 * sum_d x[t, d] in fp32, slot-aligned like xq,
            # replicated across the 128 partitions
            cvals = np.zeros((nch * CS,), dtype=np.float32)
            for c in range(nch):
                lo, hi = offs[c], min(offs[c + 1], counts[e])
                if hi > lo:
                    cvals[c * CS : c * CS + (hi - lo)] = (
                        0.5 * x[toks[lo:hi]].sum(axis=1)
                    )
            im["cq"] = np.ascontiguousarray(
                np.broadcast_to(
                    cvals.reshape(nch, 1, CS), (nch, 128, CS)
                )
            )
        in_maps.append(im)

    res = run_bass_kernel_spmd(
        nc,
        in_maps,
        list(range(N_CORES)),
        trace=os.environ.get("BASS_TRACE") == "1",
    )
    global last_results
    last_results = res

    out = np.empty((T, D), dtype=np.float32)
    for e in range(E):
        toks = order[starts[e] : starts[e + 1]]
        out[toks] = res.results[e]["yT"][:, : counts[e]].T
    if res.exec_time_ns is not None:
        print(f"HW exec time: {res.exec_time_ns} ns")
    return out[:, None, :]

